# revision 37
# baseline (speedup 1.0000x reference)
"""MoE grouped-MLP (Megatron GroupedMLP fwd, no gate) on 8 TRN2 NeuronCores.

Strategy (F8 tensor-parallel): every core processes ALL 8192 tokens; the
FFN dimension F=4096 is split 8 ways (fh=512 per core), for every expert.
Token axis is walked in expert-pure ragged tiles (identical on all cores),
so per-core compute is perfectly balanced at the 8-core floor.

Each core emits a partial fc2 output (its fh-slice of the F contraction)
in bf16; the host sums the 8 partials in fp32.  All matmuls run transposed
(fc1^T = w1^T @ x^T, out^T = w2^T @ act^T) so both weight operands load
in their natural [K, M] layouts and no on-device transposes are needed.

v2 pipeline structure (vs the all-weights-resident v1):
  - Weights are STREAMED per expert run (2 MB working set, double
    buffered) instead of 16.8 MB resident, freeing SBUF for deeper x/out
    pools.  Weight DMAs drain from a queue, ~one 0.5 MB piece per tile
    slot, so they never crowd out x loads.
  - fc2 for tile t is emitted AFTER fc1 of tile t+1 (one-slot software
    pipeline), giving the gelu a full tile of slack — PE never waits on
    the Activation engine in steady state.
  - fc1 runs m-outer (gelu per m fires 1/4-tile early, so the next tile's
    fc1 PSUM reuse is WAR-free) except tile 0, which runs k-outer to
    stream behind its chunked x/w1 prologue DMAs, borrowing the (still
    idle) fc2 PSUM pool.
  - Tiles run in expert order, large remainders first (their ragged
    stores overlap later compute); the last expert has a medium remainder
    (>=256 keeps store chunks >=512 B) for a short, DMA-dense tail.
  - Out stores ride the gpsimd SWDGE ring, split in half per tile; the
    final tile's quarters alternate over the two (by then idle) HWDGE
    rings so the end-of-program completion chain is minimal.
"""

from collections import deque
from contextlib import ExitStack

import ml_dtypes
import numpy as np

import concourse.bass as bass
import concourse.mybir as mybir
import concourse.tile as tile
from concourse import bacc
from concourse.bass_utils import run_bass_kernel_spmd

NTILE = 512  # max token tile (moving-operand free dim; one fp32 PSUM bank)
BF16 = mybir.dt.bfloat16
F8E3 = mybir.dt.float8e3  # e3m4: x rides as fp8 (moving operand streams at
F32 = mybir.dt.float32    # bf16 rate regardless) -> half the x DMA bytes.
NP_BF16 = ml_dtypes.bfloat16
NP_F8E3 = ml_dtypes.float8_e3m4

E, H, F, T = 8, 1024, 4096, 8192
FH = F // 8  # per-core ffn slice
OUT_DT = BF16  # partial-output dtype (host sums 8 partials in fp32)

_NC_CACHE = {}


def _build(tiles, h, fh):
    """Trace the SPMD bass program: one core's fh-slice of the full MLP.

    tiles: tuple of (expert, token_offset, width) in processing order,
    grouped into expert-contiguous runs (identical on all cores).  An
    expert may appear in more than one run (used by the repeat bench).
    """
    key = (tiles, h, fh)
    if key in _NC_CACHE:
        return _NC_CACHE[key]

    p_tok = T
    kh = h // 128    # fc1 contraction tiles
    kf = fh // 128   # fc2 contraction tiles
    m1 = fh // 128   # fc1 output partition tiles
    m2 = h // 128    # fc2 output partition tiles

    # expert-contiguous runs; weights are streamed per RUN
    runs = []  # (expert, first_ti)
    run_of = []
    for ti, (e, _, _) in enumerate(tiles):
        if ti == 0 or tiles[ti - 1][0] != e:
            runs.append((e, ti))
        run_of.append(len(runs) - 1)

    nc = bacc.Bacc()
    xT = nc.dram_tensor("xT", [h, p_tok], F8E3, kind="ExternalInput")
    w1h = nc.dram_tensor("w1h", [E, h, fh], BF16, kind="ExternalInput")
    w2h = nc.dram_tensor("w2h", [E, fh, h], BF16, kind="ExternalInput")
    outT = nc.dram_tensor("outT", [h, p_tok], OUT_DT, kind="ExternalOutput")

    # k-major partition views: row (128k + p) -> [p, k, cols]
    xT_v = xT.rearrange("(k p) t -> p k t", p=128)
    outT_v = outT.rearrange("(m p) t -> p m t", p=128)

    with tile.TileContext(nc) as tc, ExitStack() as ctx:
        wpool = ctx.enter_context(tc.tile_pool(name="weights", bufs=2))
        xpool = ctx.enter_context(tc.tile_pool(name="x", bufs=5))
        apool = ctx.enter_context(tc.tile_pool(name="act", bufs=3))
        opool = ctx.enter_context(tc.tile_pool(name="out", bufs=3))
        ps1 = ctx.enter_context(tc.tile_pool(name="ps1", bufs=4, space="PSUM"))
        ps2 = ctx.enter_context(tc.tile_pool(name="ps2", bufs=4, space="PSUM"))

        w1_sb, w2_sb = {}, {}

        def alloc_w1(r):
            t1 = wpool.tile([128, kh, fh], BF16, name=f"w1_r{r}", tag="w1")
            w1_sb[r] = t1
            return t1

        def w1_piece(r, c, nchunks):
            # chunk c of run r's w1, along the contraction axis
            w1v = w1h[runs[r][0]].rearrange("(k p) f -> p k f", p=128)
            step = kh // nchunks
            nc.sync.dma_start(
                out=w1_sb[r][:, c * step : (c + 1) * step, :],
                in_=w1v[:, c * step : (c + 1) * step, :],
            )

        def alloc_w2(r):
            t2 = wpool.tile([128, kf, h], BF16, name=f"w2_r{r}", tag="w2")
            w2_sb[r] = t2
            return t2

        def w2_piece(r, c, nchunks):
            # chunk c of run r's w2, along output columns (m order)
            w2v = w2h[runs[r][0]].rearrange("(k p) f -> p k f", p=128)
            step = h // nchunks
            nc.sync.dma_start(
                out=w2_sb[r][:, :, c * step : (c + 1) * step],
                in_=w2v[:, :, c * step : (c + 1) * step],
            )

        wq = deque()

        def enqueue_run(r):
            # tile allocation happens when the FIRST piece is emitted so the
            # pool ring advances in emission order
            def w1first(r=r):
                alloc_w1(r)
                w1_piece(r, 0, 2)

            wq.append(w1first)
            wq.append(lambda r=r: w1_piece(r, 1, 2))

            def w2first(r=r):
                alloc_w2(r)
                w2_piece(r, 0, 2)

            wq.append(w2first)
            wq.append(lambda r=r: w2_piece(r, 1, 2))

        x_sb = {}

        def load_x(ti):
            e, off, w = tiles[ti]
            xt = xpool.tile([128, kh, w], F8E3, name="x", tag="x")
            nc.scalar.dma_start(out=xt, in_=xT_v[:, :, off : off + w])
            x_sb[ti] = xt

        # ---- PE warm-up: the HAM clock gate holds the PE at 1.2 GHz until
        # ~3.4 us of sustained matmul activity.  The PE is idle during the
        # prologue DMAs anyway, so burn that window on dummy matmuls over
        # memset scratch — the first real matmuls then run at 2.4 GHz.
        warm = xpool.tile([128, NTILE], BF16, name="warm", tag="warm")
        nc.vector.memset(warm, 0)
        for _ in range(10):
            wps = ps1.tile([128, NTILE], F32, name="warmps", tag="fc1ps")
            nc.tensor.matmul(wps, warm[:, :128], warm, start=True, stop=True)

        # ---- prologue: interleave w1[run0] chunks with x0 chunks, finest
        # pieces first, so the first matmul starts after ~1/8 of each lands
        # and tile 0's k-outer fc1 streams behind the rest.
        t1_first = alloc_w1(0)
        w1v0 = w1h[runs[0][0]].rearrange("(k p) f -> p k f", p=128)
        xt0 = xpool.tile([128, kh, tiles[0][2]], F8E3, name="x", tag="x")
        off0, w0 = tiles[0][1], tiles[0][2]
        for lo, hi in ((0, 1), (1, 2), (2, 4), (4, 6), (6, 8)):
            nc.sync.dma_start(out=t1_first[:, lo:hi, :], in_=w1v0[:, lo:hi, :])
            nc.scalar.dma_start(
                out=xt0[:, lo:hi, :], in_=xT_v[:, lo:hi, off0 : off0 + w0]
            )
        x_sb[0] = xt0
        if len(tiles) > 1:
            load_x(1)
        alloc_w2(0)
        w2_piece(0, 0, 4)
        wq.append(lambda: w2_piece(0, 1, 4))
        wq.append(lambda: w2_piece(0, 2, 4))
        wq.append(lambda: w2_piece(0, 3, 4))
        if len(runs) > 1:
            enqueue_run(1)
        next_run_to_enqueue = 2

        pending = {}

        def emit_fc2(ti, nstores=1, rings=None, cuts=None):
            e, off, w = tiles[ti]
            acts, r = pending.pop(ti)
            rings = rings or [nc.gpsimd]
            ot = opool.tile([128, m2, w], OUT_DT, name="o", tag="o")
            if cuts is None:
                mstep = m2 // nstores
                cuts = list(range(mstep, m2 + 1, mstep))
            prev = 0
            for m in range(m2):
                ps = ps2.tile([128, w], F32, name="fc2ps", tag="fc2ps")
                for k in range(kf):
                    nc.tensor.matmul(
                        ps,
                        w2_sb[r][:, k, 128 * m : 128 * (m + 1)],
                        acts[k],
                        start=(k == 0),
                        stop=(k == kf - 1),
                    )
                nc.vector.tensor_copy(ot[:, m, :], ps)
                if m + 1 in cuts:
                    ring = rings[cuts.index(m + 1) % len(rings)]
                    ring.dma_start(
                        out=outT_v[:, prev : m + 1, off : off + w],
                        in_=ot[:, prev : m + 1, :],
                    )
                    prev = m + 1

        for ti, (e, off, w) in enumerate(tiles):
            r = run_of[ti]
            # first tile of a new run -> enqueue the NEXT run's weights
            if ti > 0 and run_of[ti - 1] != r:
                if r + 1 < len(runs) and r + 1 >= next_run_to_enqueue:
                    enqueue_run(r + 1)
                    next_run_to_enqueue = r + 2
            if ti == 0 and len(tiles) > 2:
                load_x(2)
            if ti + 3 < len(tiles):
                load_x(ti + 3)
            # drain queued weight pieces: ~one per slot, two when backlogged
            for _ in range(2 if len(wq) > 3 else 1):
                if wq:
                    wq.popleft()()

            # fc1.  Tile 0 runs k-outer (all m1 PSUM banks accumulate
            # together) so it consumes the x tile in k order, streaming
            # behind its chunked DMA.  Later tiles run m-outer so each m's
            # gelu fires 1/4-tile early — the next tile's fc1 never waits on
            # the Activation engine to release its PSUM bank.
            xt = x_sb.pop(ti)
            acts = []
            if ti == 0:
                # borrow the (still idle) fc2 PSUM pool for tile 0 so tile
                # 1's fc1 allocations from ps1 start WAR-free; fc2(t0) would
                # wait on gelu(t0) via the act dependency anyway.
                pss = [
                    ps2.tile([128, w], F32, name="fc2ps", tag="fc2ps")
                    for _ in range(m1)
                ]
                for k in range(kh):
                    for m in range(m1):
                        nc.tensor.matmul(
                            pss[m],
                            w1_sb[r][:, k, 128 * m : 128 * (m + 1)],
                            xt[:, k, :],
                            start=(k == 0),
                            stop=(k == kh - 1),
                        )
                for m in range(m1):
                    a = apool.tile([128, w], BF16, name=f"a_{m}", tag=f"a{m}")
                    nc.scalar.activation(a, pss[m], mybir.ActivationFunctionType.Gelu)
                    acts.append(a)
            else:
                for m in range(m1):
                    ps = ps1.tile([128, w], F32, name="fc1ps", tag="fc1ps")
                    for k in range(kh):
                        nc.tensor.matmul(
                            ps,
                            w1_sb[r][:, k, 128 * m : 128 * (m + 1)],
                            xt[:, k, :],
                            start=(k == 0),
                            stop=(k == kh - 1),
                        )
                    a = apool.tile([128, w], BF16, name=f"a_{m}", tag=f"a{m}")
                    nc.scalar.activation(a, ps, mybir.ActivationFunctionType.Gelu)
                    acts.append(a)
            pending[ti] = (acts, r)

            if ti >= 1:
                emit_fc2(ti - 1, nstores=2)
        emit_fc2(len(tiles) - 1, rings=[nc.scalar, nc.sync], cuts=[4, 7, 8])

    nc.compile()  # bacc legalization: splits multi-wait DMAs for TRN2 codegen
    _NC_CACHE[key] = nc
    return nc


def _plan(tokens_per_expert, reps=1):
    """Expert-contiguous ragged tiles over the token axis.

    Experts are processed in descending-remainder order (ragged stores get
    maximal overlap with later compute) except the last slot, which goes to
    an expert with a medium remainder (>=256 keeps the final store's DMA
    chunks >=512 B; small keeps the fc2+store tail short).
    """
    tpe = np.asarray(tokens_per_expert, dtype=np.int64)
    offs = np.concatenate([[0], np.cumsum(tpe)])
    n = len(tpe)
    rem = tpe % NTILE
    last_w = np.where(rem > 0, rem, NTILE)
    # candidates with 256 <= last tile < 512; fall back to global min width
    cand = [e for e in range(n) if 256 <= last_w[e] < NTILE]
    last_e = min(cand, key=lambda e: last_w[e]) if cand else int(np.argmin(last_w))
    order = sorted([e for e in range(n) if e != last_e], key=lambda e: -last_w[e])
    order.append(last_e)

    tiles = []
    for e in order:
        off, left = int(offs[e]), int(tpe[e])
        while left > 0:
            w = min(NTILE, left)
            tiles.append((e, off, w))
            off += w
            left -= w
    return tpe, tuple(tiles * reps)


def prepare(dispatched_input, tokens_per_expert, w1, w2, reps=1):
    """Build (nc, in_maps, gather) for the F8 tensor-parallel SPMD program."""
    t_tot, h = dispatched_input.shape
    n_exp, _, f = w1.shape
    fh = f // 8
    tpe, tiles = _plan(tokens_per_expert, reps=reps)

    nc = _build(tiles, h, fh)

    xT = np.ascontiguousarray(dispatched_input.astype(NP_F8E3).T)
    in_maps = []
    for c in range(8):
        fs = slice(c * fh, (c + 1) * fh)
        # Cast per-core slices directly: one pass over the fp32 weights
        # instead of full-array cast + slice copy.
        in_maps.append(
            {
                "xT": xT,
                "w1h": w1[:, :, fs].astype(NP_BF16),
                "w2h": w2[:, fs, :].astype(NP_BF16),
            }
        )

    def gather(per_core_out):
        # Two independent accumulators pipeline the bf16->f32 casts better.
        a = per_core_out[0].astype(np.float32)
        b = per_core_out[1].astype(np.float32)
        for c in range(2, 8, 2):
            a += per_core_out[c].astype(np.float32)
            b += per_core_out[c + 1].astype(np.float32)
        a += b
        return a.T

    return nc, in_maps, gather


def kernel(dispatched_input, tokens_per_expert, w1, w2, _spmd_kwargs=None):
    nc, in_maps, gather = prepare(dispatched_input, tokens_per_expert, w1, w2)
    res = run_bass_kernel_spmd(
        nc, in_maps, core_ids=list(range(8)), **(_spmd_kwargs or {})
    )
    global LAST_RESULT
    LAST_RESULT = res
    return gather([r["outT"] for r in res.results])
